# revision 3
# baseline (speedup 1.0000x reference)
"""CheckersGPT dense transformer forward pass on 8 Trainium2 NeuronCores.

Strategy: pure data-parallel over the batch dim (16 batches -> 2 per core).
Each core runs the full 6-layer transformer on its 512 tokens (2 batches x
256 tokens) with all weights replicated. No collectives needed; the final
[2, 512] probability slices are concatenated on the host.

Algebraic folding (host-side, fp32):
  A_h = Wq_h @ Wk_h.T   so  energy = x A x^T   (no K projection on device)
  B_h = Wv_h @ Wo_h     so  attn_out = sum_h (att_h @ x) B_h  (no V projection)
  LN1 affine folded into ff1 (h1 feeds only the FFN),
  last-layer LN2 affine folded into wout.

Numerics: matmul operands are bf16 (weights pre-converted on host; activation
operands rounded to bf16 on the PSUM->SBUF evacuation), accumulation is fp32
in PSUM, and all softmax / layernorm / residual math is fp32.

Layout convention per core (P=128 partitions):
  xT   [128, 4, 512]  : x transposed; chunk c holds embed dims [128c,128c+128),
                        free dim = 512 tokens. Used as matmul lhsT/rhs.
  xN   [128, 4, 512]  : x natural; chunk c holds tokens [128c,128c+128),
                        free dim = 512 embed. Used for residuals / LN (fp32).
  xB   [128, 4, 512]  : bf16 copy of xN; matmul lhsT for att@x.
All matmuls are out = lhsT.T @ rhs with contraction on the partition dim.
The last layer only computes q/attention/FFN for the final token of each
batch (the head reads only x[:, -1, :]).
"""

import os
import numpy as np
from contextlib import ExitStack

import ml_dtypes
import concourse.bass as bass
import concourse.tile as tile
from concourse import bacc, mybir
from concourse.bass_utils import run_bass_kernel_spmd

F32 = mybir.dt.float32
BF16 = mybir.dt.bfloat16
I32 = mybir.dt.int32
AX = mybir.AxisListType
ALU = mybir.AluOpType
ACTF = mybir.ActivationFunctionType

V, E, L, H, B, T = 512, 512, 6, 8, 16, 256
NCORES = 8
BPC = B // NCORES          # batches per core
TOK = BPC * T              # tokens per core
P = 128
EC = E // P                # embed chunks of 128
TC = TOK // P              # token chunks of 128
NEG = -1e9
EPS = 1e-5

MODE = os.environ.get("CKGPT_MM_DT", "bf16")   # bf16 | f32r | f32
MM_DT = {"bf16": BF16, "f32r": F32, "f32": F32}[MODE]
MM_CAST = mybir.dt.float32r if MODE == "f32r" else None
NP_WDT = ml_dtypes.bfloat16 if MODE == "bf16" else np.float32

_CACHE = {}


def _c(ap):
    """Cast an AP for matmul input (f32r mode only)."""
    return ap.bitcast(MM_CAST) if MM_CAST is not None else ap


def _mm(nc, out, lhsT, rhs, start, stop):
    nc.tensor.matmul(out, _c(lhsT), _c(rhs), start=start, stop=stop)


def _build(nlayers=L, reps=1, last_opt=True):
    nc = bacc.Bacc("TRN2", target_bir_lowering=False, debug=False, num_devices=NCORES)

    def din(name, shape, dtype=F32):
        return nc.dram_tensor(name, list(shape), dtype, kind="ExternalInput").ap()

    tok = din("tok", [P, TC], I32)            # token ids, p-major within chunks
    emb = din("emb", [V, E])
    pe2 = din("pe2", [TOK, E])                # positional encoding tiled over BPC
    wa = din("wa", [L, H, E, E], MM_DT)       # A_h = Wq Wk^T
    wb = din("wb", [L, H, E, E], MM_DT)       # B_h = Wv Wo_h
    bo = din("bo", [L, E])
    ln2w = din("ln2w", [L, E])
    ln2b = din("ln2b", [L, E])
    ff1w = din("ff1w", [L, E, E], MM_DT)      # LN1 affine folded in
    ff1b = din("ff1b", [L, E])
    ff2w = din("ff2w", [L, E, E], MM_DT)
    ff2b = din("ff2b", [L, E])
    wout = din("wout", [E, V], MM_DT)         # last-layer LN2 affine folded in
    bout = din("bout", [V])
    masks = din("masks", [2, P, T])           # additive causal mask per i-chunk
    ident = din("ident", [P, P])
    probs = nc.dram_tensor("probs", [BPC, V], F32, kind="ExternalOutput").ap()
    aps = (emb, pe2, wa, wb, bo, ln2w, ln2b,
           ff1w, ff1b, ff2w, ff2b, wout, bout, masks, ident, probs, tok)

    with tile.TileContext(nc) as tc, ExitStack() as ctx:
        if reps > 1:
            with tc.For_i(0, reps, 1):
                _emit(nc, tc, ctx, aps, nlayers, last_opt)
        else:
            _emit(nc, tc, ctx, aps, nlayers, last_opt)

    nc.compile()
    return nc


def _emit(nc, tc, ctx, aps, nlayers, last_opt):
    (emb, pe2, wa, wb, bo, ln2w, ln2b,
     ff1w, ff1b, ff2w, ff2b, wout, bout, masks, ident, probs, tok) = aps
    ep = ctx.enter_context

    const = ep(tc.tile_pool(name="const", bufs=1))
    wa_p = ep(tc.tile_pool(name="wa", bufs=3))
    wb_p = ep(tc.tile_pool(name="wb", bufs=3))
    wff_p = ep(tc.tile_pool(name="wff", bufs=1))
    bias_p = ep(tc.tile_pool(name="bias", bufs=1))
    act_p = ep(tc.tile_pool(name="act", bufs=2))
    qkv_p = ep(tc.tile_pool(name="qkvact", bufs=3))
    ot_p = ep(tc.tile_pool(name="ot", bufs=3))
    ff_p = ep(tc.tile_pool(name="ffact", bufs=3))
    tmp_p = ep(tc.tile_pool(name="tmp", bufs=3))
    esb_p = ep(tc.tile_pool(name="esb", bufs=6))
    attT_p = ep(tc.tile_pool(name="attT", bufs=3))
    st_p = ep(tc.tile_pool(name="stats", bufs=8))
    out_p = ep(tc.tile_pool(name="outp", bufs=1))

    ppb = ep(tc.tile_pool(name="ppb", bufs=3, space="PSUM"))
    ppa = ep(tc.tile_pool(name="ppa", bufs=3, space="PSUM"))
    ppt = ep(tc.tile_pool(name="ppt", bufs=2, space="PSUM"))

    # ---- constants ----
    ident_t = const.tile([P, P], F32)
    nc.sync.dma_start(out=ident_t[:], in_=ident)
    mask_t = const.tile([P, 2, T], F32)
    nc.sync.dma_start(out=mask_t[:], in_=masks.rearrange("c p j -> p c j"))
    eps_t = const.tile([P, 1], F32)
    nc.vector.memset(eps_t[:], EPS)
    tok_t = const.tile([P, TC], I32)
    nc.sync.dma_start(out=tok_t[:], in_=tok)

    def wtile(pool, dram2d, tag):
        t = pool.tile([P, EC, E], MM_DT, tag=tag)
        nc.sync.dma_start(
            out=_c(t[:]),
            in_=_c(dram2d.rearrange("(c p) o -> p c o", p=P)),
        )
        return t

    def bbcast(vec_ap, tag="bias"):
        t = bias_p.tile([P, E], F32, tag=tag)
        nc.sync.dma_start(out=t[:], in_=vec_ap.partition_broadcast(P))
        return t

    def evac(dst, src, use_act):
        """PSUM -> SBUF copy (dtype conversion happens on write)."""
        if use_act:
            nc.scalar.copy(_c(dst), src)
        else:
            nc.vector.tensor_copy(_c(dst), src)

    def transpose_into(dstT, srcN, nsrc_chunks=TC):
        # dstT[:, bb, a*P:(a+1)*P] = srcN[:, a, bb*P:(bb+1)*P].T
        for a in range(nsrc_chunks):
            for bb in range(EC):
                tp = ppt.tile([P, P], F32, tag="tp")
                nc.tensor.transpose(
                    tp[:], srcN[:, a, bb * P : (bb + 1) * P], ident_t[:]
                )
                evac(dstT[:, bb, a * P : (a + 1) * P], tp[:], (a + bb) % 2)

    def layernorm(src, dst, w_b, b_b, tag, rows=P):
        # dst = (src - mean)/sqrt(var+eps) [* w + b] ; src/dst [rows, E] fp32
        stt = st_p.tile([P, 6], F32, tag=tag + "s")
        nc.vector.bn_stats(out=stt[:rows], in_=src)
        mv = st_p.tile([P, 2], F32, tag=tag + "m")
        nc.vector.bn_aggr(out=mv[:rows], in_=stt[:rows])
        sd = st_p.tile([P, 1], F32, tag=tag + "d")
        nc.scalar.activation(
            out=sd[:rows], in_=mv[:rows, 1:2], func=ACTF.Sqrt, bias=eps_t[:rows, 0:1]
        )
        rs = st_p.tile([P, 1], F32, tag=tag + "r")
        nc.vector.reciprocal(out=rs[:rows], in_=sd[:rows])
        nc.vector.tensor_scalar(
            out=dst, in0=src, scalar1=mv[:rows, 0:1], scalar2=rs[:rows],
            op0=ALU.subtract, op1=ALU.mult,
        )
        if w_b is not None:
            nc.gpsimd.tensor_mul(out=dst, in0=dst, in1=w_b[:rows, :])
            nc.gpsimd.tensor_add(out=dst, in0=dst, in1=b_b[:rows, :])

    # ---- embedding gather + positional encoding ----
    xN = act_p.tile([P, TC, E], F32, tag="xN")
    for c in range(TC):
        nc.gpsimd.indirect_dma_start(
            out=xN[:, c, :], out_offset=None, in_=emb,
            in_offset=bass.IndirectOffsetOnAxis(ap=tok_t[:, c : c + 1], axis=0),
        )
    pe_t = ff_p.tile([P, TC, E], F32, tag="ff32")
    nc.sync.dma_start(out=pe_t[:], in_=pe2.rearrange("(c p) o -> p c o", p=P))
    for c in range(TC):
        nc.vector.tensor_add(out=xN[:, c, :], in0=xN[:, c, :], in1=pe_t[:, c, :])
    xT = act_p.tile([P, EC, TOK], MM_DT, tag="xT")
    transpose_into(xT, xN)
    xB = act_p.tile([P, TC, E], MM_DT, tag="xB")
    for c in range(TC):
        nc.scalar.copy(_c(xB[:, c, :]), xN[:, c, :])

    for l in range(nlayers):
        last = last_opt and (l == L - 1) and (nlayers == L)
        bo_b = bbcast(bo[l], "b_bo")
        if not last:
            ln2w_b = bbcast(ln2w[l], "b_l2w")
            ln2b_b = bbcast(ln2b[l], "b_l2b")
        ff2b_b = bbcast(ff2b[l], "b_f2")
        ff1b_t = bias_p.tile([P, EC], F32, tag="b_f1")
        nc.sync.dma_start(out=ff1b_t[:], in_=ff1b[l].rearrange("(c p) -> p c", p=P))

        if not last:
            attn_acc = act_p.tile([P, TC, E], F32, tag="acc")
        else:
            pw_last = ppb.tile([BPC, E], F32, tag="ppb")

        for h in range(H):
            wa_t = wtile(wa_p, wa[l, h], "wa")
            wb_t = wtile(wb_p, wb[l, h], "wb")

            if not last:
                # Q'^T = A^T x^T : [e'-chunk, tokens]
                QT = qkv_p.tile([P, EC, TOK], MM_DT, tag="qkv")
                for oc in range(EC):
                    ps = ppb.tile([P, TOK], F32, tag="ppb")
                    for ec in range(EC):
                        _mm(nc, ps[:], wa_t[:, ec, oc * P : (oc + 1) * P],
                            xT[:, ec, :], ec == 0, ec == EC - 1)
                    evac(QT[:, oc, :], ps[:], True)

                for b in range(BPC):
                    t0 = b * T
                    # --- energy + softmax, causal-skipped ---
                    # i-chunk 0 attends j in [0,128); i-chunk 1 attends [0,256)
                    pse0 = ppa.tile([P, T], F32, tag="ppa")
                    for ec in range(EC):
                        _mm(nc, pse0[:, 0:P], QT[:, ec, (2 * b) * P : (2 * b + 1) * P],
                            xT[:, ec, t0 : t0 + P], ec == 0, ec == EC - 1)
                    pse1 = ppa.tile([P, T], F32, tag="ppa")
                    for ec in range(EC):
                        _mm(nc, pse1[:], QT[:, ec, (2 * b + 1) * P : (2 * b + 2) * P],
                            xT[:, ec, t0 : t0 + T], ec == 0, ec == EC - 1)

                    att0 = esb_p.tile([P, P], F32, tag="esb0")
                    nc.vector.tensor_add(
                        out=att0[:], in0=pse0[:, 0:P], in1=mask_t[:, 0, 0:P]
                    )
                    att1 = esb_p.tile([P, T], F32, tag="esb1")
                    nc.vector.tensor_add(out=att1[:], in0=pse1[:], in1=mask_t[:, 1, :])
                    for att in (att0, att1):
                        nmax = st_p.tile([P, 1], F32, tag="nmax")
                        nc.vector.reduce_max(
                            out=nmax[:], in_=att[:], axis=AX.X, negate=True
                        )
                        den = st_p.tile([P, 1], F32, tag="den")
                        nc.scalar.activation(
                            out=att[:], in_=att[:], func=ACTF.Exp,
                            bias=nmax[:, 0:1], accum_out=den[:],
                        )
                        rec = st_p.tile([P, 1], F32, tag="rec")
                        nc.vector.reciprocal(out=rec[:], in_=den[:])
                        nc.gpsimd.tensor_scalar_mul(
                            out=att[:], in0=att[:], scalar1=rec[:]
                        )

                    # --- transpose att blocks: (i0,j0), (i1,j0), (i1,j1) ---
                    attT = attT_p.tile([P, 2, T], MM_DT, tag="attT")
                    blocks = [(att0, 0, 0, 0), (att1, 0, 0, P), (att1, P, 1, P)]
                    for k, (src, joff, jc, ioff) in enumerate(blocks):
                        tp = ppt.tile([P, P], F32, tag="tp")
                        nc.tensor.transpose(
                            tp[:], src[:, joff : joff + P], ident_t[:]
                        )
                        evac(attT[:, jc, ioff : ioff + P], tp[:], k % 2)

                    # --- att @ x : o'^T[e,i] accumulated over j-chunks ---
                    oTb = ot_p.tile([P, EC, T], MM_DT, tag="oT")
                    for ec in range(EC):
                        po = ppa.tile([P, T], F32, tag="ppa")
                        _mm(nc, po[:], xB[:, 2 * b, ec * P : (ec + 1) * P],
                            attT[:, 0, :], True, False)
                        _mm(nc, po[:, P:T], xB[:, 2 * b + 1, ec * P : (ec + 1) * P],
                            attT[:, 1, P:T], False, True)
                        evac(oTb[:, ec, :], po[:], False)

                    # --- B_h partial for this (h, b) ---
                    for tcl in range(2):
                        tcc = 2 * b + tcl
                        pw = ppb.tile([P, E], F32, tag="ppb")
                        for ec in range(EC):
                            _mm(nc, pw[:], oTb[:, ec, tcl * P : (tcl + 1) * P],
                                wb_t[:, ec, :], ec == 0, ec == EC - 1)
                        if h == 0:
                            nc.vector.tensor_copy(attn_acc[:, tcc, :], pw[:])
                        else:
                            nc.vector.tensor_add(
                                out=attn_acc[:, tcc, :],
                                in0=attn_acc[:, tcc, :], in1=pw[:],
                            )
            else:
                # ---- last layer: only the final token of each batch ----
                QTl = qkv_p.tile([P, EC, BPC], MM_DT, tag="qtl")
                for oc in range(EC):
                    ps = ppt.tile([P, BPC], F32, tag="tp")
                    for ec in range(EC):
                        _mm(nc, ps[:], wa_t[:, ec, oc * P : (oc + 1) * P],
                            xT[:, ec, T - 1 :: T], ec == 0, ec == EC - 1)
                    evac(QTl[:, oc, :], ps[:], True)
                oTl = ot_p.tile([P, EC, BPC], MM_DT, tag="oTl")
                for b in range(BPC):
                    t0 = b * T
                    pse = ppa.tile([1, T], F32, tag="ppa")
                    for ec in range(EC):
                        _mm(nc, pse[:], QTl[:, ec, b : b + 1],
                            xT[:, ec, t0 : t0 + T], ec == 0, ec == EC - 1)
                    att = esb_p.tile([1, T], F32, tag="esbl")
                    nmax = st_p.tile([1, 1], F32, tag="nmaxl")
                    nc.vector.reduce_max(
                        out=nmax[:], in_=pse[:], axis=AX.X, negate=True
                    )
                    den = st_p.tile([1, 1], F32, tag="denl")
                    nc.scalar.activation(
                        out=att[:], in_=pse[:], func=ACTF.Exp,
                        bias=nmax[0:1, 0:1], accum_out=den[:],
                    )
                    rec = st_p.tile([1, 1], F32, tag="recl")
                    nc.vector.reciprocal(out=rec[:], in_=den[:])
                    nc.vector.tensor_scalar_mul(out=att[:], in0=att[:], scalar1=rec[:])
                    attTl = attT_p.tile([P, 2, 1], MM_DT, tag="attTl")
                    for jc in range(2):
                        tp = ppt.tile([P, 1], F32, tag="tp")
                        nc.tensor.transpose(
                            tp[:], att[0:1, jc * P : (jc + 1) * P],
                            ident_t[0:1, 0:1],
                        )
                        evac(attTl[:, jc, 0:1], tp[:], jc % 2)
                    for ec in range(EC):
                        po = ppt.tile([P, 1], F32, tag="tp")
                        _mm(nc, po[:], xB[:, 2 * b, ec * P : (ec + 1) * P],
                            attTl[:, 0, 0:1], True, False)
                        _mm(nc, po[:], xB[:, 2 * b + 1, ec * P : (ec + 1) * P],
                            attTl[:, 1, 0:1], False, True)
                        evac(oTl[:, ec, b : b + 1], po[:], False)
                # accumulate B_h partials directly in PSUM across (h, ec)
                for ec in range(EC):
                    _mm(nc, pw_last[:], oTl[:, ec, :], wb_t[:, ec, :],
                        h == 0 and ec == 0, h == H - 1 and ec == EC - 1)

        # ---- FFN block ----
        ff1w_t = wtile(wff_p, ff1w[l], "wff1")
        ff2w_t = wtile(wff_p, ff2w[l], "wff2")

        if not last:
            h1N = ff_p.tile([P, TC, E], F32, tag="ff32")
            for tcc in range(TC):
                nc.gpsimd.tensor_add(
                    out=attn_acc[:, tcc, :], in0=attn_acc[:, tcc, :], in1=bo_b[:]
                )
                s1 = tmp_p.tile([P, E], F32, tag="s1")
                nc.gpsimd.tensor_add(
                    out=s1[:], in0=attn_acc[:, tcc, :], in1=xN[:, tcc, :]
                )
                layernorm(s1[:], h1N[:, tcc, :], None, None, "ln1")

            h1T = ff_p.tile([P, EC, TOK], MM_DT, tag="ffT")
            transpose_into(h1T, h1N)

            r1T = ff_p.tile([P, EC, TOK], MM_DT, tag="ffT")
            for fc in range(EC):
                ps = ppb.tile([P, TOK], F32, tag="ppb")
                for ec in range(EC):
                    _mm(nc, ps[:], ff1w_t[:, ec, fc * P : (fc + 1) * P],
                        h1T[:, ec, :], ec == 0, ec == EC - 1)
                nc.scalar.activation(
                    out=_c(r1T[:, fc, :]), in_=ps[:], func=ACTF.Relu,
                    bias=ff1b_t[:, fc : fc + 1],
                )

            xN_new = act_p.tile([P, TC, E], F32, tag="xN")
            for tcc in range(TC):
                ps = ppb.tile([P, E], F32, tag="ppb")
                for fc in range(EC):
                    _mm(nc, ps[:], r1T[:, fc, tcc * P : (tcc + 1) * P],
                        ff2w_t[:, fc, :], fc == 0, fc == EC - 1)
                s2 = tmp_p.tile([P, E], F32, tag="s1")
                nc.vector.tensor_add(out=s2[:], in0=ps[:], in1=ff2b_b[:])
                nc.gpsimd.tensor_add(out=s2[:], in0=s2[:], in1=attn_acc[:, tcc, :])
                layernorm(s2[:], xN_new[:, tcc, :], ln2w_b, ln2b_b, "ln2")
            xN = xN_new
            xT = act_p.tile([P, EC, TOK], MM_DT, tag="xT")
            transpose_into(xT, xN)
            xB = act_p.tile([P, TC, E], MM_DT, tag="xB")
            for c in range(TC):
                nc.scalar.copy(_c(xB[:, c, :]), xN[:, c, :])
        else:
            # ---- last layer FFN on 2 tokens only ----
            ao_l = out_p.tile([BPC, E], F32)
            nc.vector.tensor_add(out=ao_l[:], in0=pw_last[:], in1=bo_b[:BPC, :])
            x_l = out_p.tile([BPC, E], F32)
            for b in range(BPC):
                nc.sync.dma_start(
                    out=x_l[b : b + 1, :], in_=xN[P - 1 : P, 2 * b + 1, :]
                )
            s1 = out_p.tile([BPC, E], F32)
            nc.vector.tensor_add(out=s1[:], in0=ao_l[:], in1=x_l[:])
            h1_l = out_p.tile([BPC, E], F32)
            layernorm(s1[:], h1_l[:], None, None, "lnL1", rows=BPC)
            h1T_l = ff_p.tile([P, EC, BPC], MM_DT, tag="h1Tl")
            for bb in range(EC):
                tp = ppt.tile([P, BPC], F32, tag="tp")
                nc.tensor.transpose(
                    tp[:], h1_l[:, bb * P : (bb + 1) * P],
                    ident_t[0:BPC, 0:BPC],
                )
                evac(h1T_l[:, bb, :], tp[:], bb % 2)
            r1T_l = ff_p.tile([P, EC, BPC], MM_DT, tag="r1Tl")
            for fc in range(EC):
                ps = ppt.tile([P, BPC], F32, tag="tp")
                for ec in range(EC):
                    _mm(nc, ps[:], ff1w_t[:, ec, fc * P : (fc + 1) * P],
                        h1T_l[:, ec, :], ec == 0, ec == EC - 1)
                nc.scalar.activation(
                    out=_c(r1T_l[:, fc, :]), in_=ps[:], func=ACTF.Relu,
                    bias=ff1b_t[:, fc : fc + 1],
                )
            ps2 = ppb.tile([BPC, E], F32, tag="ppb")
            for fc in range(EC):
                _mm(nc, ps2[:], r1T_l[:, fc, :], ff2w_t[:, fc, :],
                    fc == 0, fc == EC - 1)
            s2 = out_p.tile([BPC, E], F32)
            nc.vector.tensor_add(out=s2[:], in0=ps2[:], in1=ff2b_b[:BPC, :])
            nc.vector.tensor_add(out=s2[:], in0=s2[:], in1=ao_l[:])
            xl = out_p.tile([BPC, E], F32)
            layernorm(s2[:], xl[:], None, None, "lnL2", rows=BPC)
            xlT = ff_p.tile([P, EC, BPC], MM_DT, tag="xlT")
            for bb in range(EC):
                tp = ppt.tile([P, BPC], F32, tag="tp")
                nc.tensor.transpose(
                    tp[:], xl[:, bb * P : (bb + 1) * P],
                    ident_t[0:BPC, 0:BPC],
                )
                evac(xlT[:, bb, :], tp[:], bb % 2)

    # ---- output head: last token of each batch ----
    wout_t = wtile(wb_p, wout, "wb")
    bout_t = out_p.tile([BPC, V], F32)
    nc.sync.dma_start(out=bout_t[:], in_=bout.partition_broadcast(BPC))
    pl = ppb.tile([BPC, V], F32, tag="ppb")
    if nlayers == L and last_opt:
        xl_lhs = xlT
        cols = slice(0, BPC)
    else:
        xl_lhs = xT
        cols = slice(T - 1, TOK, T)
    for ec in range(EC):
        _mm(nc, pl[:], xl_lhs[:, ec, cols], wout_t[:, ec, :], ec == 0, ec == EC - 1)
    logits = out_p.tile([BPC, V], F32)
    nc.vector.tensor_add(out=logits[:], in0=pl[:], in1=bout_t[:])
    nmax = out_p.tile([BPC, 1], F32)
    nc.vector.reduce_max(out=nmax[:], in_=logits[:], axis=AX.X, negate=True)
    den = out_p.tile([BPC, 1], F32)
    nc.scalar.activation(
        out=logits[:], in_=logits[:], func=ACTF.Exp,
        bias=nmax[:, 0:1], accum_out=den[:],
    )
    rec = out_p.tile([BPC, 1], F32)
    nc.vector.reciprocal(out=rec[:], in_=den[:])
    nc.vector.tensor_scalar_mul(out=logits[:], in0=logits[:], scalar1=rec[:])
    nc.sync.dma_start(out=probs, in_=logits[:])


def _pe_table():
    i = np.arange(E, dtype=np.float32)
    rates = (1.0 / np.power(np.float32(10000.0), 2.0 * np.floor(i / 2.0) / E)).astype(
        np.float32
    )
    ang = np.arange(T, dtype=np.float32)[:, None] * rates[None, :]
    pe = np.concatenate([np.sin(ang[:, 0::2]), np.cos(ang[:, 1::2])], axis=-1)
    return np.tile(pe.astype(np.float32), (BPC, 1))  # [TOK, E]


def _masks():
    m = np.zeros((2, P, T), dtype=np.float32)
    j = np.arange(T)
    for c in range(2):
        i = c * P + np.arange(P)
        m[c] = np.where(j[None, :] > i[:, None], np.float32(NEG), np.float32(0.0))
    return m


def _prep_in_maps(
    input_tokens, emb, wq, wk, wv, wo, bo, ln1_w, ln1_b, ln2_w, ln2_b,
    ff1_w, ff1_b, ff2_w, ff2_b, wout, bout,
):
    f = lambda x: np.ascontiguousarray(np.asarray(x, dtype=np.float32))
    w = lambda x: np.ascontiguousarray(np.asarray(x, dtype=np.float32).astype(NP_WDT))
    toks = np.asarray(input_tokens).astype(np.int64)

    wqf = np.asarray(wq, dtype=np.float32)
    wkf = np.asarray(wk, dtype=np.float32)
    wvf = np.asarray(wv, dtype=np.float32)
    wof = np.asarray(wo, dtype=np.float32).reshape(L, H, E, E)
    # A_h = Wq_h Wk_h^T ; B_h = Wv_h Wo_h
    wa_np = np.einsum("lheo,lhfo->lhef", wqf, wkf, optimize=True)
    wb_np = np.matmul(wvf, wof)
    # fold LN1 affine into ff1 (h1 only feeds the FFN)
    l1w = np.asarray(ln1_w, dtype=np.float32)
    l1b = np.asarray(ln1_b, dtype=np.float32)
    f1w = np.asarray(ff1_w, dtype=np.float32)
    f1b = np.asarray(ff1_b, dtype=np.float32)
    ff1w_f = l1w[:, :, None] * f1w
    ff1b_f = np.einsum("le,leo->lo", l1b, f1w) + f1b
    # fold last-layer LN2 affine into wout
    l2w = np.asarray(ln2_w, dtype=np.float32)
    l2b = np.asarray(ln2_b, dtype=np.float32)
    woutf = np.asarray(wout, dtype=np.float32)
    wout_f = l2w[L - 1][:, None] * woutf
    bout_f = l2b[L - 1] @ woutf + np.asarray(bout, dtype=np.float32)

    shared = {
        "emb": f(emb), "wa": w(wa_np), "wb": w(wb_np),
        "bo": f(bo), "ln2w": f(l2w), "ln2b": f(l2b),
        "ff1w": w(ff1w_f), "ff1b": f(ff1b_f), "ff2w": w(ff2_w),
        "ff2b": f(ff2_b), "wout": w(wout_f), "bout": f(bout_f),
        "pe2": _pe_table(), "masks": _masks(),
        "ident": np.eye(P, dtype=np.float32),
    }
    in_maps = []
    for c in range(NCORES):
        t = toks[c * BPC : (c + 1) * BPC].reshape(TOK)  # [512] flat tokens
        tokarr = np.ascontiguousarray(t.reshape(TC, P).T.astype(np.int32))
        in_maps.append({**shared, "tok": tokarr})
    return in_maps


def kernel(**inputs):
    if "nc" not in _CACHE:
        _CACHE["nc"] = _build()
    nc = _CACHE["nc"]
    in_maps = _prep_in_maps(**inputs)
    res = run_bass_kernel_spmd(nc, in_maps, core_ids=list(range(NCORES)))
    _CACHE["last_results"] = res
    out = np.concatenate([res.results[c]["probs"] for c in range(NCORES)], axis=0)
    return out.astype(np.float32)
